# revision 44
# baseline (speedup 1.0000x reference)
"""Conv2D 3x3 (stride 1, pad 1) Bass kernel for Trainium2, 8 NeuronCores.

Problem: x (32,128,56,56) f32, Wk (256,128,3,3) f32, b (256,) f32
         -> out (32,256,56,56) f32

Strategy:
  - Data-parallel over batch: 4 images per core, 8 cores. No collectives.
  - Implicit GEMM: 9 shifted matmuls (one per filter tap) accumulate in PSUM.
    Contraction dim = in_c = 128 (exactly the partition dim).
  - x staged in SBUF with spatial zero-padding to 58x58 so every tap is a
    pure strided slice (no boundary fixups).
  - Output tiled as [oc_chunk(128) x 8 rows x 56 cols] = 448-wide free dim
    per matmul (one PSUM bank, fp32 accumulate).
  - x/W cast to fp16 on host: PE streams 1 col/cycle with the weight load
    hidden (~189 ns/MM vs 211 for float32r), rel err ~2.7e-4 vs fp32 ref.
  - Bias (kept fp32) added during PSUM->SBUF evacuation on the DVE.
Measured: ~113.4 us HW exec per NEFF, cool chip (513 matmuls/core gap-free at the
PE issue-rate floor; staging/output DMA and bias-add fully overlapped).
"""

import os

import numpy as np

import concourse.bacc as bacc
import concourse.bass as bass
import concourse.mybir as mybir
from concourse.bass_utils import run_bass_kernel_spmd
from concourse.tile import TileContext

B, IN_C, OUT_C, H, W, KS = 32, 128, 256, 56, 56, 3
N_CORES = 8
B_PER = B // N_CORES          # 4 images per core
HP, WP = H + 2, W + 2          # 58 padded
RB = 8                         # output rows per matmul block
N_RB = H // RB                 # 7 row blocks
P = 128
OC_CHUNKS = OUT_C // P         # 2

# matmul input dtype.
#   float16: 1 cycle/row, weight load hidden (FWL) -> ~189 ns/MM, rel err ~3e-4
#   float32r: fp32-accurate (~1.4e-4) but ~211 ns/MM (in-instruction weight
#             stream is not hidden) and 2x the staging DMA bytes
_MM_DTYPE_NAME = os.environ.get("CONV_MM_DTYPE", "float16")
_MM_DTYPE = getattr(mybir.dt, _MM_DTYPE_NAME)
_MM_NP_DTYPE = {"float16": np.float16, "float32r": np.float32, "bfloat16": None}[
    _MM_DTYPE_NAME
]
if _MM_NP_DTYPE is None:
    import ml_dtypes

    _MM_NP_DTYPE = ml_dtypes.bfloat16


def _build_program():
    f32 = mybir.dt.float32
    # Bacc (not raw Bass): its finalize() runs the wait-splitting passes
    # (each TRN2 instruction can carry at most one sync wait).
    nc = bacc.Bacc("TRN2", target_bir_lowering=False)

    x_ext = nc.declare_dram_parameter(
        "x", [IN_C, B_PER, HP, WP], _MM_DTYPE, isOutput=False
    )
    w_ext = nc.declare_dram_parameter("w", [IN_C, KS * KS, OUT_C], _MM_DTYPE, isOutput=False)
    b_ext = nc.declare_dram_parameter("b", [P, OC_CHUNKS], f32, isOutput=False)
    o_ext = nc.declare_dram_parameter("out", [B_PER, OUT_C, H, W], f32, isOutput=True)

    with TileContext(nc) as tc:
        with (
            tc.tile_pool(name="const", bufs=1) as cpool,
            tc.tile_pool(name="psum", bufs=7, space="PSUM") as ppool,
            tc.tile_pool(name="warmp", bufs=1, space="PSUM") as dwarm,
            tc.tile_pool(name="outp", bufs=8) as opool,
        ):
            x_sb = cpool.tile([IN_C, B_PER, HP, WP], _MM_DTYPE, name="x_sb")
            w_sb = cpool.tile([IN_C, KS * KS, OUT_C], _MM_DTYPE, name="w_sb")
            b_sb = cpool.tile([P, OC_CHUNKS], f32, name="b_sb")

            # Staging: one HWDGE queue moves ~92 GB/s and each DMA trigger
            # costs ~0.6us serialized on the Sync queue, so chunk sizes and
            # trigger order are chosen so the first matmuls' data (tap
            # weights + first rows of image 0) lands earliest, and later
            # chunks stay ahead of the PE's row consumption. Chunks land on
            # distinct HW queues (round-robin), so transfers overlap.
            # First-tile deps (x rows 0:10, taps in consumption order) are
            # interleaved across BOTH trigger engines: tap 0 fires first on
            # Scalar while the x head fires on Sync, so the first matmul's
            # inputs all land by ~9.3us and later taps pace its consumption.
            nc.sync.dma_start(out=x_sb[:, 0, 0:5], in_=x_ext[:, 0, 0:5])
            nc.scalar.dma_start(out=w_sb[:, 0:1], in_=w_ext[:, 0:1])
            nc.sync.dma_start(out=x_sb[:, 0, 5:10], in_=x_ext[:, 0, 5:10])
            nc.scalar.dma_start(out=w_sb[:, 1:3], in_=w_ext[:, 1:3])
            nc.sync.dma_start(out=w_sb[:, 5:7], in_=w_ext[:, 5:7])
            nc.scalar.dma_start(out=w_sb[:, 3:5], in_=w_ext[:, 3:5])
            nc.sync.dma_start(out=w_sb[:, 7:9], in_=w_ext[:, 7:9])
            nc.scalar.dma_start(out=b_sb[:], in_=b_ext[:])
            for r0, r1 in [(10, 22), (22, 34), (34, 46), (46, 58)]:
                nc.sync.dma_start(out=x_sb[:, 0, r0:r1], in_=x_ext[:, 0, r0:r1])
            for n in range(1, B_PER):
                for r0, r1 in [(0, 29), (29, 58)]:
                    nc.sync.dma_start(
                        out=x_sb[:, n, r0:r1], in_=x_ext[:, n, r0:r1]
                    )

            # Pre-warm the PE HAM clock gate during the dead staging window:
            # ~3.4us of matmul activity flips the PE from 1.2 to 2.4 GHz, so
            # burn dummy matmuls on an (uninitialized-content, never-read)
            # scratch tile while the first x/w chunks are still in flight.
            # No DMA deps -> these start right after the kernel preamble.
            warm_sb = cpool.tile([P, 128], mybir.dt.bfloat16, name="warm_sb")
            warm_ps = dwarm.tile([P, 128], f32, name="warm_ps")
            nc.vector.memset(warm_sb[:], 0)
            for i in range(30):
                nc.tensor.matmul(
                    warm_ps[:],
                    lhsT=warm_sb[:],
                    rhs=warm_sb[:],
                    start=(i == 0),
                    stop=False,
                    skip_group_check=True,
                )

            def emit_tile(n, ci, row0, nrows, store):
                ps = ppool.tile([P, nrows, W], f32, name="ps", tag="ps")
                t = 0
                for kh in range(KS):
                    for kw in range(KS):
                        nc.tensor.matmul(
                            ps[:],
                            lhsT=w_sb[:, kh * KS + kw, ci * P : (ci + 1) * P],
                            rhs=x_sb[:, n, row0 + kh : row0 + kh + nrows, kw : kw + W],
                            start=(t == 0),
                            stop=(t == KS * KS - 1),
                        )
                        t += 1
                ot = opool.tile([P, nrows, W], f32, name="ot", tag="ot")
                # explicit DVE: nc.any routes this to ScalarE, which is
                # ~9x slower for plain copy+add and becomes the bottleneck
                nc.vector.tensor_scalar_add(ot[:], ps[:], b_sb[:, ci : ci + 1])
                # output triggers go out on the (otherwise idle) Scalar
                # HWDGE queue, keeping Sync free for staging. The final
                # tile splits across two queues so the last transfer
                # doesn't serialize into the tail.
                o_dst = o_ext[n, ci * P : (ci + 1) * P, row0 : row0 + nrows, :]
                if store == "split":
                    # Sync is idle by the end of the run while Scalar still
                    # drains its trigger backlog — issue the two halves from
                    # different engines so they trigger + transfer in parallel
                    h = nrows // 2
                    nc.sync.dma_start(out=o_dst[:, 0:h], in_=ot[:, 0:h])
                    nc.scalar.dma_start(out=o_dst[:, h:nrows], in_=ot[:, h:nrows])
                elif store == "sync":
                    nc.sync.dma_start(out=o_dst, in_=ot[:])
                else:
                    nc.scalar.dma_start(out=o_dst, in_=ot[:])

            for n in range(B_PER):
                for rb in range(N_RB):
                    for ci in range(OC_CHUNKS):
                        last_rb = n == B_PER - 1 and rb == N_RB - 1
                        if last_rb and ci == OC_CHUNKS - 1:
                            # the final tile runs as three shrinking pieces on
                            # alternating store engines so evacuation + store
                            # overlap the tail instead of serializing entirely
                            # after the last matmul (same total PE columns)
                            emit_tile(n, ci, rb * RB + 0, 3, "scalar")
                            emit_tile(n, ci, rb * RB + 3, 3, "sync")
                            emit_tile(n, ci, rb * RB + 6, 2, "scalar")
                        elif last_rb:
                            emit_tile(n, ci, rb * RB, RB, "split")
                        else:
                            emit_tile(n, ci, rb * RB, RB, "scalar")
    nc.finalize()  # Bacc.finalize runs the wait-splitting compile passes
    return nc


_NC_CACHE = {}


def _get_program():
    if "nc" not in _NC_CACHE:
        _NC_CACHE["nc"] = _build_program()
    return _NC_CACHE["nc"]


def _prep_inputs(x, Wk, b):
    x = np.asarray(x, dtype=np.float32)
    Wk = np.asarray(Wk, dtype=np.float32)
    b = np.asarray(b, dtype=np.float32)
    # [oc, ic, kh, kw] -> [ic, kh*kw, oc]
    w_prep = np.ascontiguousarray(
        Wk.reshape(OUT_C, IN_C, KS * KS).transpose(1, 2, 0).astype(_MM_NP_DTYPE)
    )
    b_prep = np.ascontiguousarray(b.reshape(OC_CHUNKS, P).T)
    # [b, ic, h, w] -> per-core [ic, b_per, 58, 58] zero-padded
    x_pad = np.zeros((B, IN_C, HP, WP), dtype=_MM_NP_DTYPE)
    x_pad[:, :, 1 : H + 1, 1 : W + 1] = x
    in_maps = []
    for c in range(N_CORES):
        shard = np.ascontiguousarray(
            x_pad[c * B_PER : (c + 1) * B_PER].transpose(1, 0, 2, 3)
        )
        in_maps.append({"x": shard, "w": w_prep, "b": b_prep})
    return in_maps


def run(x, Wk, b, **spmd_kwargs):
    """Run the conv on 8 cores; returns (full_output, BassKernelResults)."""
    nc = _get_program()
    in_maps = _prep_inputs(x, Wk, b)
    try:
        res = run_bass_kernel_spmd(nc, in_maps, list(range(N_CORES)), **spmd_kwargs)
    except Exception:
        # transient NRT device errors have been observed to recover on retry
        import time

        time.sleep(2.0)
        res = run_bass_kernel_spmd(nc, in_maps, list(range(N_CORES)), **spmd_kwargs)
    out = np.concatenate([res.results[i]["out"] for i in range(N_CORES)], axis=0)
    return out, res


def kernel(x, Wk, b):
    out, _ = run(x, Wk, b)
    return out
